# revision 1
# baseline (speedup 1.0000x reference)
"""Trainium2 Bass kernel for nn_AxialAttentionBlock (B=2, C=768, H=W=128, 12 heads).

Sharding (8 cores):
  Kernel A (head-parallel): core c -> batch b=c//4, heads 3*(c%4)..+2.
    rms_inorm(x) folded into qkv conv weights; q/k layernorm; row+col attention
    (bf16 inner matmuls, fp32 softmax stats); norm2 scale; emits channel-major
    attn_s [192, 16384] fp32 per core.
  Kernel B (pixel-parallel): core c -> b=c//4, pixel quarter. out-conv (f32r) +
    residual -> x5; MLP (bf16) -> h2 + per-channel partial stats.
  Host reduces mlpnorm stats (tiny);
  Kernel C: out = x5 + (gamma_mlp*mlpnorm_w/(std+eps)) * h2.
"""
import math
import os
import sys

sys.path.insert(0, "/opt/trn_rl_repo")

import numpy as np
import ml_dtypes

import concourse.bass as bass
import concourse.bacc as bacc
import concourse.mybir as mybir
import concourse.tile as tile
from concourse.bass import ts, ds
from concourse.masks import make_identity

F32 = mybir.dt.float32
F32R = mybir.dt.float32r
BF16 = mybir.dt.bfloat16
AF = mybir.ActivationFunctionType
OP = mybir.AluOpType
AX = mybir.AxisListType

NUM_BUCKETS = 32
MAX_DISTANCE = 128
EPS_IN = 1e-8
EPS_LN = 1e-5
NUM_HEADS = 12
C = 768
HD = 64

PROFILE = False
LAST_TIMES = {}
_BF = ml_dtypes.bfloat16


def _rel_bias_np(L, emb):
    """numpy mirror of reference rel_bias -> [heads, L, L] f32."""
    pos = np.arange(L)
    rp = pos[None, :] - pos[:, None]
    n = NUM_BUCKETS // 2
    ret = (rp > 0).astype(np.int32) * n
    arp = np.abs(rp)
    max_exact = n // 2
    is_small = arp < max_exact
    rpf = np.maximum(arp, 1).astype(np.float32)
    val_large = max_exact + (
        np.log(rpf / max_exact) / np.float32(math.log(MAX_DISTANCE / max_exact)) * (n - max_exact)
    ).astype(np.int32)
    val_large = np.minimum(val_large, n - 1)
    bucket = ret + np.where(is_small, arp, val_large)
    return np.ascontiguousarray(emb[bucket].transpose(2, 0, 1)).astype(np.float32)


# ---------------------------------------------------------------------------
# Kernel A builder
# ---------------------------------------------------------------------------
def build_kernel_A(S=128, qn_generic=False, kn_generic=False, qkb_nonzero=False,
                   vb_nonzero=False, skip_waves=False, skip_conv=False):
    """Head-sharded attention kernel. S = H = W (128 real, smaller for sim)."""
    PX = S * S
    T = PX // 512          # px tiles of 512
    NCH = 4                # 128-px chunks per tile
    NB = S // 8            # seq blocks of 8
    FT = min(4096, PX)     # stats tile width
    NFT = PX // FT

    nc = bacc.Bacc()
    xb = nc.dram_tensor("xb", [C, PX], F32, kind="ExternalInput")
    wqkT = nc.dram_tensor("wqkT", [C, 384], F32, kind="ExternalInput")
    wvT = nc.dram_tensor("wvT", [C, 192], F32, kind="ExternalInput")
    n1w = nc.dram_tensor("n1w", [C], F32, kind="ExternalInput")
    n2w = nc.dram_tensor("n2w", [192], F32, kind="ExternalInput")
    biasT = nc.dram_tensor("biasT", [3, S, S], BF16, kind="ExternalInput")
    # generic layernorm affine (broadcast tiles); only read when flags set
    qkw_b = nc.dram_tensor("qkw_b", [128, 384], F32, kind="ExternalInput")
    qkb_b = nc.dram_tensor("qkb_b", [128, 384], F32, kind="ExternalInput")
    inb_qk = nc.dram_tensor("inb_qk", [128, 384], F32, kind="ExternalInput")
    inb_v = nc.dram_tensor("inb_v", [256], F32, kind="ExternalInput")
    attn_s = nc.dram_tensor("attn_s", [192, PX], F32, kind="ExternalOutput")

    spill = nc.dram_tensor("spill", [64, 3, PX], BF16)

    xb_r = xb.rearrange("(o p) f -> p o f", p=128)

    with tile.TileContext(nc) as tc:
        with tc.tile_pool(name="persist", bufs=1) as pp:
            ident = pp.tile([128, 128], BF16)
            make_identity(nc, ident[:])
            bias_t = [pp.tile([S, S], BF16, name=f"bias{g}") for g in range(3)]
            for g in range(3):
                nc.sync.dma_start(bias_t[g][:], biasT[g])
            n1w_t = pp.tile([128, 6], F32)
            nc.sync.dma_start(n1w_t[:], n1w.rearrange("(o p) -> p o", p=128))
            s1 = pp.tile([128, 6], F32)
            wqk_s = pp.tile([128, 6, 384], BF16)
            wv_s = pp.tile([128, 6, 192], BF16)
            qkw_t = qkb_t = inbqk_t = inbv_t = None
            if qn_generic or kn_generic:
                qkw_t = pp.tile([128, 384], F32)
                qkb_t = pp.tile([128, 384], F32)
                nc.sync.dma_start(qkw_t[:], qkw_b[:])
                nc.sync.dma_start(qkb_t[:], qkb_b[:])
            if qkb_nonzero:
                inbqk_t = pp.tile([128, 384], F32)
                nc.sync.dma_start(inbqk_t[:], inb_qk[:])
            if vb_nonzero:
                inbv_t = pp.tile([128, 2], F32)
                nc.sync.dma_start(inbv_t[:], inb_v.rearrange("(o p) -> p o", p=128))

            # ---------------- phase 1: x stats -> s1 (bn_stats) ----------------
            NST = PX // 512
            with tc.tile_pool(name="xstat", bufs=8) as sp, \
                 tc.tile_pool(name="xbn", bufs=1) as bnp:
                bnall = bnp.tile([128, 6, NST, 6], F32)
                for o in range(6):
                    for t in range(NST):
                        xt = sp.tile([128, 512], F32)
                        nc.gpsimd.dma_start(xt[:], xb_r[:, o, ts(t, 512)])
                        nc.vector.bn_stats(bnall[:, o, t, :], xt[:])
                st2 = pp.tile([128, 6, 2], F32)
                for o in range(6):
                    nc.vector.bn_aggr(st2[:, o, :], bnall[:, o])
                m2 = pp.tile([128, 6], F32)
                nc.vector.tensor_scalar_mul(m2[:], st2[:, :, 1], float(PX) / (PX - 1))
                nc.scalar.sqrt(m2[:], m2[:])
                nc.vector.tensor_scalar_add(m2[:], m2[:], EPS_IN)
                nc.vector.reciprocal(m2[:], m2[:])
                nc.vector.tensor_tensor(s1[:], m2[:], n1w_t[:], OP.mult)

            # ---------------- phase 2: fold s1 into weights ----------------
            with tc.tile_pool(name="wraw", bufs=1) as wrp:
                wqk_raw = wrp.tile([128, 6, 384], F32)
                wv_raw = wrp.tile([128, 6, 192], F32)
                nc.gpsimd.dma_start(wqk_raw[:], wqkT.rearrange("(o p) f -> p o f", p=128))
                nc.gpsimd.dma_start(wv_raw[:], wvT.rearrange("(o p) f -> p o f", p=128))
                nc.vector.tensor_tensor(wqk_s[:], wqk_raw[:],
                                        s1[:, :, None].to_broadcast((128, 6, 384)), OP.mult)
                nc.vector.tensor_tensor(wv_s[:], wv_raw[:],
                                        s1[:, :, None].to_broadcast((128, 6, 192)), OP.mult)

            # ---------------- phase 3: conv + ln + transpose ----------------
            with tc.tile_pool(name="wave0", bufs=1) as w0:
                qA = w0.tile([128, PX], BF16)
                kA = w0.tile([128, PX], BF16)
                vA = w0.tile([128, PX], BF16)
                conv_rng = range(0) if skip_conv else range(T)
                with tc.tile_pool(name="cxt", bufs=3) as cxt, \
                     tc.tile_pool(name="cps", bufs=2, space="PSUM") as cps, \
                     tc.tile_pool(name="cpsv", bufs=2, space="PSUM") as cpsv, \
                     tc.tile_pool(name="ctp", bufs=2, space="PSUM") as ctp, \
                     tc.tile_pool(name="csm", bufs=3) as csm, \
                     tc.tile_pool(name="cst", bufs=2) as cst:
                    for t in conv_rng:
                        xt = cxt.tile([128, 6, 512], F32)
                        nc.gpsimd.dma_start(xt[:], xb_r[:, :, ts(t, 512)])
                        xt16 = cxt.tile([128, 6, 512], BF16, tag="xt16")
                        nc.gpsimd.tensor_copy(xt16[:], xt[:])
                        xtr = xt16[:]
                        stage2 = cst.tile([64, 3, 512], BF16)
                        # v conv (channel-major)
                        for mo, msz in ((0, 128), (1, 64)):
                            psv = cpsv.tile([128, 512], F32)
                            for k in range(6):
                                nc.tensor.matmul(
                                    psv[0:msz, :],
                                    wv_s[:, k, ds(mo * 128, msz)],
                                    xtr[:, k, :],
                                    start=(k == 0), stop=(k == 5))
                            if vb_nonzero:
                                nc.scalar.activation(
                                    psv[0:msz, :], psv[0:msz, :], AF.Identity,
                                    bias=inbv_t[0:msz, mo:mo + 1])
                            if mo == 0:
                                nc.vector.tensor_copy(vA[:, ts(t, 512)], psv[:])
                            else:
                                nc.vector.tensor_copy(stage2[:, 2, :], psv[0:64, :])
                        # qk conv (pixel-major) per 128-px chunk
                        for ci in range(NCH):
                            psqk = cps.tile([128, 384], F32)
                            for k in range(6):
                                nc.tensor.matmul(
                                    psqk[:], xtr[:, k, ds(ci * 128, 128)],
                                    wqk_s[:, k, :],
                                    start=(k == 0), stop=(k == 5))
                            qksb = csm.tile([128, 384], F32, tag="qksb")
                            if qkb_nonzero:
                                nc.vector.tensor_tensor(qksb[:], psqk[:], inbqk_t[:], OP.add)
                            else:
                                nc.vector.tensor_copy(qksb[:], psqk[:])
                            scr = csm.tile([128, 384], F32, tag="sqscr")
                            nc.vector.scalar_tensor_tensor(scr[:], qksb[:], 1.0, qksb[:],
                                                           OP.mult, OP.mult)
                            ssq_c = csm.tile([128, 6], F32, tag="ssq")
                            sums_c = csm.tile([128, 6], F32, tag="ssum")
                            nc.vector.tensor_reduce(
                                ssq_c[:], scr[:].rearrange("p (g d) -> p g d", d=64),
                                AX.X, OP.add)
                            nc.vector.tensor_reduce(
                                sums_c[:], qksb[:].rearrange("p (g d) -> p g d", d=64),
                                AX.X, OP.add)
                            # ln stats: mean, rstd -> scale/bias
                            mean_c = csm.tile([128, 6], F32, tag="mean")
                            var_c = csm.tile([128, 6], F32, tag="var")
                            m2_c = csm.tile([128, 6], F32, tag="m2c")
                            nc.vector.tensor_scalar_mul(mean_c[:], sums_c[:], 1.0 / 64)
                            nc.vector.tensor_scalar_mul(var_c[:], ssq_c[:], 1.0 / 64)
                            nc.vector.tensor_tensor(m2_c[:], mean_c[:], mean_c[:], OP.mult)
                            nc.vector.scalar_tensor_tensor(var_c[:], m2_c[:], -1.0, var_c[:],
                                                           OP.mult, OP.add)
                            nc.vector.tensor_scalar_add(var_c[:], var_c[:], EPS_LN)
                            nc.scalar.sqrt(var_c[:], var_c[:])
                            nc.vector.reciprocal(var_c[:], var_c[:])        # rstd
                            scale_c = var_c
                            bias_c = csm.tile([128, 6], F32, tag="bias")
                            nc.vector.scalar_tensor_tensor(bias_c[:], mean_c[:], -1.0, scale_c[:],
                                                           OP.mult, OP.mult)
                            # fold 1/sqrt(hd) into k half
                            nc.vector.tensor_scalar_mul(scale_c[:, 3:6], scale_c[:, 3:6], 0.125)
                            nc.vector.tensor_scalar_mul(bias_c[:, 3:6], bias_c[:, 3:6], 0.125)
                            ptq = ctp.tile([128, 128], BF16, tag="ptq")
                            ptk = ctp.tile([128, 128], BF16, tag="ptq")
                            pt2 = ctp.tile([64, 2, 128], BF16, tag="pt2")
                            for g in range(6):
                                qpm = csm.tile([128, 64], BF16, tag="qpm")
                                nc.vector.tensor_scalar(qpm[:], qksb[:, ts(g, 64)],
                                                        scale_c[:, g:g + 1],
                                                        bias_c[:, g:g + 1],
                                                        OP.mult, OP.add)
                                if (g < 3 and qn_generic) or (g >= 3 and kn_generic):
                                    nc.vector.tensor_tensor(qpm[:], qpm[:],
                                                            qkw_t[:, ts(g, 64)], OP.mult)
                                    nc.vector.tensor_tensor(qpm[:], qpm[:],
                                                            qkb_t[:, ts(g, 64)], OP.add)
                                hh = g % 3
                                if hh < 2:
                                    dst = ptq if g < 3 else ptk
                                    nc.tensor.transpose(dst[ds(64 * hh, 64), :], qpm[:], ident[:])
                                else:
                                    nc.tensor.transpose(pt2[:, 0 if g < 3 else 1, :],
                                                        qpm[:], ident[:])
                            nc.vector.tensor_copy(qA[:, ds(t * 512 + ci * 128, 128)], ptq[:])
                            nc.vector.tensor_copy(kA[:, ds(t * 512 + ci * 128, 128)], ptk[:])
                            nc.vector.tensor_copy(stage2[:, 0:2, ds(ci * 128, 128)], pt2[:])
                        nc.gpsimd.dma_start(spill[:, :, ts(t, 512)], stage2[:])

                # ---------------- phase 4: wave 0 attention (heads 0,1) -------
                xxA = w0.tile([128, PX], BF16)
                if not skip_waves:
                    _attention_wave(nc, tc, S, [0, 1], qA, kA, vA, xxA, bias_t, ident, base_of=lambda i: 64 * i)
                    _norm2_emit(nc, tc, S, xxA, 128, n2w, 0, attn_s)

            # ---------------- phase 5: wave 1 (head 2) ----------------
            if skip_waves:
                return nc
            with tc.tile_pool(name="wave1", bufs=1) as w1:
                qB = w1.tile([64, PX], BF16)
                kB = w1.tile([64, PX], BF16)
                vB = w1.tile([64, PX], BF16)
                xxB = w1.tile([64, PX], BF16)
                nc.gpsimd.dma_start(qB[:], spill[:, 0, :])
                nc.gpsimd.dma_start(kB[:], spill[:, 1, :])
                nc.gpsimd.dma_start(vB[:], spill[:, 2, :])
                _attention_wave(nc, tc, S, [2], qB, kB, vB, xxB, bias_t, ident,
                                base_of=lambda i: 0, single=True)
                _norm2_emit(nc, tc, S, xxB, 64, n2w, 128, attn_s)

    nc.compile()
    return nc


def _attention_wave(nc, tc, S, heads, qT, kT, vT, xxT, bias_t, ident, base_of, single=False):
    """Row+col attention for the given heads. qT/kT/vT/xxT hold heads at
    partition base base_of(i) (i = index within wave)."""
    NB = S // 8
    q3 = qT[:].rearrange("p (h w) -> p h w", w=S)
    k3 = kT[:].rearrange("p (h w) -> p h w", w=S)
    v3 = vT[:].rearrange("p (h w) -> p h w", w=S)
    xx3 = xxT[:].rearrange("p (h w) -> p h w", w=S)
    PB = 64 if single else 128

    NP4 = min(512 // S, 8)  # instances packed per PSUM bank (4 at S=128)
    with tc.tile_pool(name="aps", bufs=3, space="PSUM") as aps, \
         tc.tile_pool(name="apt", bufs=2, space="PSUM") as apt, \
         tc.tile_pool(name="avt", bufs=1, space="PSUM") as avt, \
         tc.tile_pool(name="aout", bufs=2, space="PSUM") as aout, \
         tc.tile_pool(name="asb", bufs=10) as asb, \
         tc.tile_pool(name="asm", bufs=4) as asm:
        for i, g in enumerate(heads):
            b0 = base_of(i)
            for dirn in range(2):   # 0 = row (seq along w), 1 = col (seq along h)
                for blk in range(NB):
                    sums8 = asm.tile([S, 8], F32)
                    Pts = []
                    for q4 in range(8 // NP4):
                        S_ps4 = aps.tile([S, NP4 * S], F32)
                        for j in range(NP4):
                            s = blk * 8 + q4 * NP4 + j
                            if dirn == 0:
                                qsl, ksl = q3[ds(b0, 64), s, :], k3[ds(b0, 64), s, :]
                            else:
                                qsl, ksl = q3[ds(b0, 64), :, s], k3[ds(b0, 64), :, s]
                            ssl = S_ps4[:, ds(j * S, S)]
                            nc.tensor.matmul(ssl, qsl, ksl, start=True, stop=False)
                            nc.tensor.matmul(ssl, ident[0:S, 0:S], bias_t[g][:],
                                             start=False, stop=True)
                        Pt4 = asb.tile([S, NP4 * S], BF16, tag="P")
                        nc.scalar.activation(Pt4[:], S_ps4[:], AF.Exp)
                        nc.vector.tensor_reduce(
                            sums8[:, ds(q4 * NP4, NP4)],
                            Pt4[:].rearrange("p (i f) -> p i f", i=NP4),
                            AX.X, OP.add)
                        Pts.append(Pt4)
                    rc8 = asm.tile([S, 8], F32, tag="rc")
                    nc.vector.reciprocal(rc8[:], sums8[:])
                    nc.vector.tensor_scalar_mul(rc8[:], rc8[:], 0.5)
                    for q4 in range(8 // NP4):
                        PT_ps = apt.tile([S, NP4 * S], BF16)
                        vt_ps = avt.tile([S, NP4 * 64], BF16)
                        o_ps = aout.tile([PB, NP4 * S], F32)
                        PTs = asb.tile([S, NP4 * S], BF16, tag="PT")
                        vts = asb.tile([S, NP4 * 64], BF16, tag="vt")
                        Pt4 = Pts[q4]
                        rb = rc8[:, ds(q4 * NP4, NP4), None].to_broadcast((S, NP4, S))
                        p3 = Pt4[:].rearrange("p (i f) -> p i f", i=NP4)
                        nc.gpsimd.tensor_tensor(p3, p3, rb, OP.mult)
                        for j in range(NP4):
                            s8 = q4 * NP4 + j
                            s = blk * 8 + s8
                            psl = Pt4[:, ds(j * S, S)]
                            nc.tensor.transpose(PT_ps[:, ds(j * S, S)], psl,
                                                ident[0:S, 0:S])
                            vsl = (v3[ds(b0, 64), s, :] if dirn == 0
                                   else v3[ds(b0, 64), :, s])
                            nc.tensor.transpose(vt_ps[:, ds(j * 64, 64)], vsl,
                                                ident[ds(b0, 64), ds(b0, 64)])
                        nc.vector.tensor_copy(PTs[:], PT_ps[:])
                        nc.vector.tensor_copy(vts[:], vt_ps[:])
                        for j in range(NP4):
                            nc.tensor.matmul(o_ps[ds(b0, 64), ds(j * S, S)],
                                             vts[:, ds(j * 64, 64)],
                                             PTs[:, ds(j * S, S)],
                                             start=True, stop=True)
                        s0 = blk * 8 + q4 * NP4
                        if dirn == 0:
                            nc.vector.tensor_copy(
                                xx3[ds(b0, 64), ds(s0, NP4), :],
                                o_ps[ds(b0, 64), :].rearrange("p (i f) -> p i f", i=NP4))
                        else:
                            ov = o_ps[ds(b0, 64), :].rearrange("p (i f) -> p f i", i=NP4)
                            nc.vector.tensor_tensor(
                                xx3[ds(b0, 64), :, ds(s0, NP4)],
                                xx3[ds(b0, 64), :, ds(s0, NP4)], ov, OP.add)


def _norm2_emit(nc, tc, S, xxT, nch, n2w, row0, attn_s):
    """Per-channel rms (ddof=1) over all pixels of xxT [nch, PX], scale by
    norm2_w/(std+eps), emit fp32 rows [row0:row0+nch] of attn_s."""
    PX = S * S
    FT = min(2048, PX)
    NFT = PX // FT
    with tc.tile_pool(name="n2scr", bufs=2) as scp, \
         tc.tile_pool(name="n2st", bufs=1) as stp, \
         tc.tile_pool(name="n2out", bufs=2) as outp:
        ssum = stp.tile([nch, NFT], F32)
        ssq = stp.tile([nch, NFT], F32)
        for t in range(NFT):
            scr = scp.tile([nch, FT], BF16)
            nc.scalar.activation(scr[:], xxT[:, ts(t, FT)], AF.Square,
                                 accum_out=ssq[:, t:t + 1])
            nc.vector.tensor_reduce(ssum[:, t:t + 1], xxT[:, ts(t, FT)], AX.X, OP.add)
        sm = stp.tile([nch, 1], F32)
        sq = stp.tile([nch, 1], F32)
        nc.vector.tensor_reduce(sm[:], ssum[:], AX.X, OP.add)
        nc.vector.tensor_reduce(sq[:], ssq[:], AX.X, OP.add)
        nc.vector.tensor_tensor(sm[:], sm[:], sm[:], OP.mult)
        nc.vector.scalar_tensor_tensor(sm[:], sm[:], -1.0 / PX, sq[:], OP.mult, OP.add)
        nc.vector.tensor_scalar_mul(sm[:], sm[:], 1.0 / (PX - 1))
        nc.scalar.sqrt(sm[:], sm[:])
        nc.vector.tensor_scalar_add(sm[:], sm[:], EPS_IN)
        nc.vector.reciprocal(sm[:], sm[:])
        n2t = stp.tile([nch, 1], F32)
        nc.gpsimd.dma_start(n2t[:], n2w[ds(row0, nch)].rearrange("(p o) -> p o", o=1))
        nc.vector.tensor_tensor(sm[:], sm[:], n2t[:], OP.mult)
        for t in range(NFT):
            stage = outp.tile([nch, FT], F32)
            nc.gpsimd.tensor_scalar_mul(stage[:], xxT[:, ts(t, FT)], sm[:])
            nc.gpsimd.dma_start(attn_s[ds(row0, nch), ts(t, FT)], stage[:])


# ---------------------------------------------------------------------------
# Kernel B builder
# ---------------------------------------------------------------------------
def build_kernel_B(PXQ=4096, ob_nonzero=False, b1_nonzero=False, b2_nonzero=False, sim_gelu=False):
    """Pixel-sharded: out-conv + residual -> x5; MLP -> h2 (+ stats)."""
    TW = 256
    TQ = PXQ // TW
    nc = bacc.Bacc()
    attn = nc.dram_tensor("attn", [C, PXQ], F32, kind="ExternalInput")
    xin = nc.dram_tensor("xin", [C, PXQ], F32, kind="ExternalInput")
    owT = nc.dram_tensor("owT", [C, C], F32, kind="ExternalInput")
    w1 = nc.dram_tensor("w1", [C, 3072], BF16, kind="ExternalInput")
    w2 = nc.dram_tensor("w2", [3072, C], BF16, kind="ExternalInput")
    gatt = nc.dram_tensor("gatt", [C], F32, kind="ExternalInput")
    outb = nc.dram_tensor("outb", [C], F32, kind="ExternalInput")
    b1 = nc.dram_tensor("b1", [3072], F32, kind="ExternalInput")
    b2 = nc.dram_tensor("b2", [C], F32, kind="ExternalInput")
    x5o = nc.dram_tensor("x5o", [C, PXQ], F32, kind="ExternalOutput")
    h2o = nc.dram_tensor("h2o", [C, PXQ], BF16, kind="ExternalOutput")
    sto = nc.dram_tensor("sto", [C, 2], F32, kind="ExternalOutput")

    attn_r = attn.rearrange("(o p) f -> p o f", p=128)
    xin_r = xin.rearrange("(o p) f -> p o f", p=128)
    x5o_r = x5o.rearrange("(o p) f -> p o f", p=128)
    h2o_r = h2o.rearrange("(o p) f -> p o f", p=128)

    with tile.TileContext(nc) as tc:
        with tc.tile_pool(name="wts", bufs=1) as wp:
            owt_raw = wp.tile([128, 6, C], F32)
            nc.sync.dma_start(owt_raw[:], owT.rearrange("(o p) f -> p o f", p=128))
            owt = wp.tile([128, 6, C], BF16)
            nc.vector.tensor_copy(owt[:], owt_raw[:])
            w1t = wp.tile([128, 6, 3072], BF16)
            nc.sync.dma_start(w1t[:], w1.rearrange("(o p) f -> p o f", p=128))
            w2t = wp.tile([128, 24, C], BF16)
            nc.sync.dma_start(w2t[:], w2.rearrange("(o p) f -> p o f", p=128))
            gat = wp.tile([128, 6], F32)
            nc.sync.dma_start(gat[:], gatt.rearrange("(o p) -> p o", p=128))
            obt = wp.tile([128, 6], F32)
            b1t = wp.tile([128, 24], F32)
            b2t = wp.tile([128, 6], F32)
            if ob_nonzero:
                nc.sync.dma_start(obt[:], outb.rearrange("(o p) -> p o", p=128))
            if b1_nonzero:
                nc.sync.dma_start(b1t[:], b1.rearrange("(o p) -> p o", p=128))
            if b2_nonzero:
                nc.sync.dma_start(b2t[:], b2.rearrange("(o p) -> p o", p=128))
            stsum = wp.tile([128, 6, TQ], F32)
            stsq = wp.tile([128, 6, TQ], F32)

            with tc.tile_pool(name="bt", bufs=2) as btp, \
                 tc.tile_pool(name="bx5", bufs=3) as bx5p, \
                 tc.tile_pool(name="bh", bufs=2) as bhp, \
                 tc.tile_pool(name="bps", bufs=4, space="PSUM") as bps, \
                 tc.tile_pool(name="bps2", bufs=2, space="PSUM") as bps2, \
                 tc.tile_pool(name="bscr", bufs=2) as bscr:
                for t in range(TQ):
                    at = btp.tile([128, 6, TW], F32, tag="attn")
                    nc.gpsimd.dma_start(at[:], attn_r[:, :, ts(t, TW)])
                    xt0 = btp.tile([128, 6, TW], F32, tag="xin")
                    nc.gpsimd.dma_start(xt0[:], xin_r[:, :, ts(t, TW)])
                    xt = btp.tile([128, 6, TW], F32, tag="xinc")
                    nc.vector.tensor_copy(xt[:], xt0[:])
                    at16 = btp.tile([128, 6, TW], BF16, tag="at16")
                    nc.vector.tensor_copy(at16[:], at[:])
                    atr = at16[:]
                    x5 = bx5p.tile([128, 6, TW], F32, tag="x5")
                    x5b = bx5p.tile([128, 6, TW], BF16, tag="x5b")
                    for mo in range(6):
                        ps = bps2.tile([128, TW], F32, tag="ocps")
                        for k in range(6):
                            nc.tensor.matmul(ps[:], owt[:, k, ts(mo, 128)],
                                             atr[:, k, :], start=(k == 0), stop=(k == 5))
                        if ob_nonzero:
                            nc.scalar.activation(ps[:], ps[:], AF.Identity,
                                                 bias=obt[:, mo:mo + 1])
                        nc.vector.scalar_tensor_tensor(x5[:, mo, :], ps[:], gat[:, mo:mo + 1],
                                                       xt[:, mo, :], OP.mult, OP.add)
                    nc.gpsimd.dma_start(x5o_r[:, :, ts(t, TW)], x5[:])
                    nc.vector.tensor_copy(x5b[:], x5[:])
                    # MLP1 + gelu -> h [128, 24, TW] bf16
                    h = bhp.tile([128, 24, TW], BF16, tag="h")
                    for mo in range(24):
                        ps = bps.tile([128, TW], F32, tag="m1ps")
                        for k in range(6):
                            nc.tensor.matmul(ps[:], w1t[:, k, ts(mo, 128)],
                                             x5b[:, k, :], start=(k == 0), stop=(k == 5))
                        if b1_nonzero:
                            nc.scalar.activation(ps[:], ps[:], AF.Identity,
                                                 bias=b1t[:, mo:mo + 1])
                        if sim_gelu:
                            sg = bhp.tile([128, TW], BF16, tag="sg")
                            nc.scalar.activation(sg[:], ps[:], AF.Sigmoid, scale=1.702)
                            nc.vector.tensor_tensor(h[:, mo, :], ps[:], sg[:], OP.mult)
                        else:
                            nc.scalar.activation(h[:, mo, :], ps[:], AF.Gelu)
                    # MLP2 -> h2 + stats
                    for oo in range(6):
                        ps = bps2.tile([128, TW], F32, tag="m2ps")
                        for k in range(24):
                            nc.tensor.matmul(ps[:], w2t[:, k, ts(oo, 128)],
                                             h[:, k, :], start=(k == 0), stop=(k == 23))
                        if b2_nonzero:
                            nc.scalar.activation(ps[:], ps[:], AF.Identity,
                                                 bias=b2t[:, oo:oo + 1])
                        h2 = bscr.tile([128, TW], BF16, tag="h2")
                        scr = bscr.tile([128, TW], BF16, tag="sscr")
                        nc.vector.tensor_copy(h2[:], ps[:])
                        nc.vector.scalar_tensor_tensor(scr[:], h2[:], 1.0, h2[:],
                                                       OP.mult, OP.mult,
                                                       accum_out=stsq[:, oo, t:t + 1])
                        nc.vector.tensor_reduce(stsum[:, oo, t:t + 1], h2[:], AX.X, OP.add)
                        nc.gpsimd.dma_start(h2o_r[:, oo, ts(t, TW)], h2[:])
                sfin = wp.tile([128, 6, 2], F32)
                nc.vector.tensor_reduce(sfin[:, :, 0:1], stsum[:], AX.X, OP.add)
                nc.vector.tensor_reduce(sfin[:, :, 1:2], stsq[:], AX.X, OP.add)
                nc.gpsimd.dma_start(sto.rearrange("(o p) f -> p o f", p=128), sfin[:])
    nc.compile()
    return nc


# ---------------------------------------------------------------------------
# Kernel C builder
# ---------------------------------------------------------------------------
def build_kernel_C(PXQ=4096):
    TQ = PXQ // 512
    nc = bacc.Bacc()
    x5i = nc.dram_tensor("x5i", [C, PXQ], F32, kind="ExternalInput")
    h2i = nc.dram_tensor("h2i", [C, PXQ], BF16, kind="ExternalInput")
    fv = nc.dram_tensor("fv", [C], F32, kind="ExternalInput")
    outo = nc.dram_tensor("outo", [C, PXQ], F32, kind="ExternalOutput")
    x5_r = x5i.rearrange("(o p) f -> p o f", p=128)
    h2_r = h2i.rearrange("(o p) f -> p o f", p=128)
    out_r = outo.rearrange("(o p) f -> p o f", p=128)
    with tile.TileContext(nc) as tc:
        with tc.tile_pool(name="cw", bufs=1) as wp, \
             tc.tile_pool(name="ct", bufs=3) as tp:
            f = wp.tile([128, 6], F32)
            nc.sync.dma_start(f[:], fv.rearrange("(o p) -> p o", p=128))
            for t in range(TQ):
                x5t = tp.tile([128, 6, 512], F32, tag="x5")
                h2t = tp.tile([128, 6, 512], BF16, tag="h2")
                ot = tp.tile([128, 6, 512], F32, tag="out")
                nc.gpsimd.dma_start(x5t[:], x5_r[:, :, ts(t, 512)])
                nc.gpsimd.dma_start(h2t[:], h2_r[:, :, ts(t, 512)])
                tmp = tp.tile([128, 6, 512], F32, tag="tmp")
                nc.vector.tensor_tensor(tmp[:], h2t[:],
                                        f[:, :, None].to_broadcast((128, 6, 512)), OP.mult)
                nc.vector.tensor_tensor(ot[:], tmp[:], x5t[:], OP.add)
                nc.gpsimd.dma_start(out_r[:, :, ts(t, 512)], ot[:])
    nc.compile()
    return nc


# ---------------------------------------------------------------------------
# Host orchestration
# ---------------------------------------------------------------------------
def _run(nc, in_maps, tag):
    from concourse.bass_utils import run_bass_kernel_spmd
    r = run_bass_kernel_spmd(nc, in_maps, list(range(8)), trace=PROFILE)
    if PROFILE:
        LAST_TIMES[tag] = r.exec_time_ns
    return r.results


def kernel(x, bcs, norm1_w, norm2_w, mlpnorm_w, gamma_att, gamma_mlp,
           in_w, in_b, out_w, out_b, qn_w, qn_b, kn_w, kn_b,
           relbias_emb, mlp_w1, mlp_b1, mlp_w2, mlp_b2):
    x = np.asarray(x, dtype=np.float32)
    B = x.shape[0]
    S = x.shape[2]
    PX = S * S
    he, hd = NUM_HEADS, HD
    f32 = lambda a: np.ascontiguousarray(np.asarray(a), dtype=np.float32)
    in_w, out_w = f32(in_w), f32(out_w)
    mlp_w1, mlp_w2 = f32(mlp_w1), f32(mlp_w2)
    norm1_w, norm2_w, mlpnorm_w = f32(norm1_w), f32(norm2_w), f32(mlpnorm_w)
    gamma_att, gamma_mlp = f32(gamma_att), f32(gamma_mlp)
    in_b, out_b = f32(in_b), f32(out_b)
    qn_w, qn_b, kn_w, kn_b = f32(qn_w), f32(qn_b), f32(kn_w), f32(kn_b)
    mlp_b1, mlp_b2 = f32(mlp_b1), f32(mlp_b2)

    bias_full = _rel_bias_np(S, f32(relbias_emb))        # [12, S, S]

    qn_generic = not np.allclose(qn_w, 1.0) or np.any(qn_b != 0)
    kn_generic = not np.allclose(kn_w, 1.0) or np.any(kn_b != 0)
    qkb_nonzero = bool(np.any(in_b[:2 * C] != 0))
    vb_nonzero = bool(np.any(in_b[2 * C:] != 0))

    # ---- kernel A ----
    ncA = build_kernel_A(S, qn_generic, kn_generic, qkb_nonzero, vb_nonzero)
    xf = x.reshape(B, C, PX)
    in_maps_A = []
    for c in range(8):
        b, hg0 = c // 4, 3 * (c % 4)
        hs = slice(hg0 * hd, (hg0 + 3) * hd)
        qrows = in_w[0 * C:1 * C][hs]          # [192, C]
        krows = in_w[1 * C:2 * C][hs]
        vrows = in_w[2 * C:3 * C][hs]
        wqkT = np.ascontiguousarray(np.concatenate([qrows, krows], 0).T)  # [C, 384]
        wvT = np.ascontiguousarray(vrows.T)                               # [C, 192]
        qkw = np.concatenate([np.tile(qn_w, 3), np.tile(kn_w * 0.125, 3)])
        qkb = np.concatenate([np.tile(qn_b, 3), np.tile(kn_b * 0.125, 3)])
        inbqk = np.concatenate([in_b[0 * C:1 * C][hs], in_b[1 * C:2 * C][hs]])
        in_maps_A.append({
            "xb": xf[b],
            "wqkT": wqkT,
            "wvT": wvT,
            "n1w": norm1_w,
            "n2w": norm2_w[hs],
            "biasT": bias_full[hg0:hg0 + 3].astype(_BF),
            "qkw_b": np.tile(qkw[None, :], (128, 1)).astype(np.float32),
            "qkb_b": np.tile(qkb[None, :], (128, 1)).astype(np.float32),
            "inb_qk": np.tile(inbqk[None, :], (128, 1)).astype(np.float32),
            "inb_v": np.pad(in_b[2 * C:3 * C][hs], (0, 64)),
        })
    resA = _run(ncA, in_maps_A, "A")
    attn_full = [np.concatenate([resA[b * 4 + j]["attn_s"] for j in range(4)], axis=0)
                 for b in range(B)]            # [C, PX] per b

    # ---- kernel B ----
    PXQ = PX // 4
    ob_nz = bool(np.any(out_b != 0))
    b1_nz = bool(np.any(mlp_b1 != 0))
    b2_nz = bool(np.any(mlp_b2 != 0))
    ncB = build_kernel_B(PXQ, ob_nz, b1_nz, b2_nz)
    owT = np.ascontiguousarray(out_w.T)
    w1b = mlp_w1.astype(_BF)
    w2b = mlp_w2.astype(_BF)
    in_maps_B = []
    for c in range(8):
        b, q = c // 4, c % 4
        sl = slice(q * PXQ, (q + 1) * PXQ)
        in_maps_B.append({
            "attn": np.ascontiguousarray(attn_full[b][:, sl]),
            "xin": np.ascontiguousarray(xf[b][:, sl]),
            "owT": owT, "w1": w1b, "w2": w2b,
            "gatt": gamma_att, "outb": out_b, "b1": mlp_b1, "b2": mlp_b2,
        })
    resB = _run(ncB, in_maps_B, "B")

    # ---- host: reduce mlpnorm stats ----
    fvecs = []
    for b in range(B):
        st = sum(resB[b * 4 + j]["sto"] for j in range(4))   # [C, 2]
        var = (st[:, 1] - st[:, 0] ** 2 / PX) / (PX - 1)
        std = np.sqrt(np.maximum(var, 0)).astype(np.float32)
        fvecs.append(gamma_mlp * mlpnorm_w / (std + EPS_IN))

    # ---- kernel C ----
    ncC = build_kernel_C(PXQ)
    in_maps_C = []
    for c in range(8):
        b = c // 4
        in_maps_C.append({
            "x5i": resB[c]["x5o"],
            "h2i": resB[c]["h2o"],
            "fv": fvecs[b].astype(np.float32),
        })
    resC = _run(ncC, in_maps_C, "C")

    out = np.empty((B, C, PX), np.float32)
    for c in range(8):
        b, q = c // 4, c % 4
        out[b][:, q * PXQ:(q + 1) * PXQ] = resC[c]["outo"]
    return out.reshape(B, C, S, S)

